# revision 46
# baseline (speedup 1.0000x reference)
"""Trainium2 Bass kernel for Atten2EquiVarApply.

out[b,n,i,d] = sum_{j,h} AA[b,n,i,j,h] * w[h,0] * h2[b,n,j,d]

Strategy: data-parallel over the 4096 (b,n) pairs, 512 per core on 8 cores.
Per (b,n) the device computes out^T[d,i] = sum_h ( C4_h^T @ AAT_h ) where
  C4_h[j,d]  = w[h]*h2[n,j,d]    (3-column stationary, nearly-free LDWEIGHTS)
  AAT_h[j,i] = AA[n,i,j,h]       (128x128 moving operand, streamed from SBUF)
accumulated over h in one PSUM bank. The host pre-transposes AA into
[block, j, h, nb, i] layout so the DMA lands j on partitions with 16 KiB
contiguous segments (line-rate).

Output path ("wide" mode): each PSUM group's [12,128] chunks are
PE-transposed to [128,12] (identity matmul), the valid [128,3] diagonal
columns staged into a 128-partition tile, and OUT written as [i, n, d] with
contiguous 768 B per-partition segments engaging all 16 SDMA engines. The
old out path (12-partition source, 512 B write descriptors on ~2 engines)
cost ~15 us/pass — it was the entire gap to the pure-stream DMA floor of
~390 us (134 MiB @ ~343 GB/s effective per-core HBM rate).
All arithmetic happens on device; host does only data movement
(transpose/tile/reshape).
"""

import os
import sys
import time

import numpy as np

for _p in ("/opt/trn_rl_repo", "/root/.axon_site/_ro/trn_rl_repo"):
    if os.path.isdir(_p) and _p not in sys.path:
        sys.path.insert(0, _p)

import concourse.bass as bass
import concourse.mybir as mybir
import concourse.tile as tile

NF, NLOC, NNEI, NH, D = 4, 1024, 128, 4, 3
NCORES = 8
NTOT = NF * NLOC          # 4096 (b,n) pairs
NPC = NTOT // NCORES      # 512 per core
NB = 8                    # (b,n) pairs per DMA block (2 MiB per block)
NBLK = NPC // NB          # blocks per core
OB = 8                    # blocks per output DMA flush
AAT_RINGS = ("sync",)     # HWDGE ring per block (round-robin)
OUT_MODE = "wide"         # OUT [i,n,d] via PE-transpose (see docstring)
F32 = mybir.dt.float32

def _split_excess_waits(nc):
    """The pinned walrus build rejects any instruction carrying more than one
    semaphore wait ("Too many sync wait commands"). Hoist the extra waits onto
    same-engine NOPs placed immediately before the instruction — per-engine
    program order makes that equivalent.
    """
    # Phase 1: create the nops (add_instruction appends them to the current
    # block's tail — not where we want them) and plan the splice points.
    nops_for = {}   # target instruction name -> [nop Instruction, ...]
    nop_names = set()
    for f in nc.m.functions:
        for bb in f.blocks:
            for ins in list(bb.instructions):
                if ins.name in nop_names:
                    continue
                si = ins.sync_info
                if si is not None and si.on_wait and len(si.on_wait) > 1:
                    waits = list(si.on_wait)
                    created = []
                    for w in waits[:-1]:
                        nop_bi = nc.engines[ins.engine].nop(nofuse=True)
                        nop_bi.ins.sync_info = mybir.SyncInfo(
                            on_wait=[w], on_update=[]
                        )
                        created.append(nop_bi.ins)
                        nop_names.add(nop_bi.ins.name)
                    si.on_wait = waits[-1:]
                    nops_for[ins.name] = created
    # Phase 2: rebuild every block, dropping the auto-appended nop copies and
    # inserting each nop immediately before its target instruction.
    for f in nc.m.functions:
        for bb in f.blocks:
            rebuilt = []
            for ins in bb.instructions:
                if ins.name in nop_names:
                    continue
                rebuilt.extend(nops_for.get(ins.name, ()))
                rebuilt.append(ins)
            bb.instructions = rebuilt
    return nc


def build_nc(NBLK=NBLK, NPC=NPC, NB=NB, OB=OB, aat_bufs=4, psum_bufs=8, obuf_bufs=3, rings=AAT_RINGS, out_eng="scalar", out_mode=OUT_MODE, repeat=1, hw_loop=0, no_out=False, no_copy=False, no_mm=False):
    """Build the per-core Bass program (identical on all 8 cores)."""
    nc = bass.Bass()
    # [block, j, h, nb, i]
    aat_d = nc.declare_dram_parameter("AAT", [NBLK, NNEI, NH, NB, NNEI], F32, isOutput=False)
    # [j, n, d] = h2[n, j, d]
    h1_d = nc.declare_dram_parameter("H1", [NNEI, NPC, D], F32, isOutput=False)
    # [j, h] = w[h] (partition-broadcast)
    ws_d = nc.declare_dram_parameter("WS", [NNEI, NH], F32, isOutput=False)
    tick_d = nc.declare_dram_parameter("tick", [128, 8], F32, isOutput=False)
    NGRP = NBLK * (NB // 4)  # matmul groups (4 n each)
    # kd:   OUT[k, d, g, i] (k = n%4, g = n//4): per (k,d) partition-row the
    #       per-window write is one contiguous ng*512B segment, but src spans
    #       only 12 SBUF partitions (few SDMA engines).
    # wide: OUT[i, n, d]: valid [3,128] PSUM blocks are PE-transposed to
    #       [128,3] so the staged output spans all 128 partitions and the
    #       flush DMA writes contiguous 768B segments from every partition.
    if out_mode == "wide":
        out_d = nc.declare_dram_parameter("OUT", [NNEI, NBLK * NB, D], F32, isOutput=True)
        id12_d = nc.declare_dram_parameter("ID12", [4 * D, 4 * D], F32, isOutput=False)
    else:
        out_d = nc.declare_dram_parameter("OUT", [4, D, NGRP, NNEI], F32, isOutput=True)
    tock_d = nc.declare_dram_parameter("tock", [128, 8], F32, isOutput=True)

    wide = out_mode == "wide"
    import contextlib
    with tile.TileContext(nc) as tc:
        with contextlib.ExitStack() as _stack:
            const_pool = _stack.enter_context(tc.tile_pool(name="const", bufs=1))
            aat_pool = _stack.enter_context(tc.tile_pool(name="aat", bufs=aat_bufs))
            psum_pool = _stack.enter_context(tc.tile_pool(
                name="psum", bufs=(4 if wide else psum_bufs), space="PSUM"))
            psum2_pool = _stack.enter_context(tc.tile_pool(
                name="psum2", bufs=4, space="PSUM")) if wide else None
            obuf_pool = _stack.enter_context(tc.tile_pool(name="obuf", bufs=obuf_bufs))
            wide_pool = _stack.enter_context(tc.tile_pool(
                name="wide", bufs=3)) if wide else None
            # tick -> tock passthrough (chain-timing dependency), DRAM->DRAM.
            # Constants + outputs ride the ACT HWDGE ring so the SP ring is a
            # pure AAT stream.
            nc.scalar.dma_start(tock_d[:], tick_d[:])

            h1_all = const_pool.tile([NNEI, NPC * D], F32)
            ws_all = const_pool.tile([NNEI, NH], F32)
            c4_all = const_pool.tile([NNEI, NPC * NH * D], mybir.dt.float32r)
            nc.scalar.dma_start(
                h1_all[:].rearrange("p (n d) -> p n d", n=NPC), h1_d[:]
            )
            nc.scalar.dma_start(ws_all[:], ws_d[:])
            if wide:
                id12 = const_pool.tile([4 * D, 4 * D], F32)
                nc.scalar.dma_start(id12[:], id12_d[:])
            # C4[:, (h, n, d)] = w[h] * h2[n, j, d] — w applied on device via a
            # per-partition scalar (all partitions hold the same w[h])
            for h in range(NH):
                nc.vector.tensor_scalar_mul(
                    c4_all[:, h * NPC * D : (h + 1) * NPC * D],
                    h1_all[:],
                    ws_all[:, h : h + 1],
                )

            GRP = 4                      # n's packed per matmul (N = GRP*128)
            GPB = NB // GRP              # groups per block
            import contextlib
            loop_cm = tc.For_i(0, hw_loop, 1) if hw_loop else contextlib.nullcontext()
            with loop_cm:
              for _rep in range(repeat):
                for b in range(NBLK):
                  aat = aat_pool.tile([NNEI, NH * NB * NNEI], mybir.dt.float32r)
                  in_eng = getattr(nc, rings[b % len(rings)])
                  in_eng.dma_start(
                      aat[:].rearrange("p (h n i) -> p h n i", h=NH, n=NB),
                      aat_d[b].bitcast(mybir.dt.float32r),
                  )
                  if b % OB == 0:
                      obuf_t = obuf_pool.tile([128, OB * GPB * GRP * NNEI], F32)
                      obuf = obuf_t[0 : GRP * D, :]
                      if wide:
                          wtile = wide_pool.tile([NNEI, OB * NB * D], F32)
                  for g in range(GPB):
                      if no_mm:
                          continue
                      n0 = g * GRP
                      ng0 = b * NB + n0
                      # block-diagonal pack: stationary [j, GRP*3] (contiguous in
                      # the h-major C4), moving [j, GRP*128]; PSUM [12, 512] is
                      # one full bank. Off-diagonal cells are garbage the host
                      # ignores.
                      ps = psum_pool.tile([GRP * D, GRP * NNEI], F32)
                      for h in range(NH):
                          # float32r view: same fp32 bytes, single-pass PE at
                          # 1 cycle/row (fp32 proper costs 4) for moving dim>=256
                          nc.tensor.matmul(
                              ps[:],
                              c4_all[:, (h * NPC + ng0) * D : (h * NPC + ng0 + GRP) * D],
                              aat[:, h * NB * NNEI + n0 * NNEI : h * NB * NNEI + (n0 + GRP) * NNEI],
                              start=(h == 0),
                              stop=(h == NH - 1),
                          )
                      gslot = (b % OB) * GPB + g
                      if not no_copy:
                          # PSUM reads must cover the tile's full partition
                          # range, so copy the whole [12,512] incl. the
                          # off-diagonal garbage; the valid diagonal blocks
                          # are extracted downstream.
                          nc.vector.tensor_copy(
                              obuf[:, gslot * GRP * NNEI : (gslot + 1) * GRP * NNEI], ps[:]
                          )
                          if wide:
                              # PE-transpose each [12,128] chunk to [128,12]
                              # (stationary base partition must be 0) and
                              # extract the valid [128,3] diagonal columns so
                              # the staged output spans all 128 partitions.
                              for k in range(GRP):
                                  ps2 = psum2_pool.tile([NNEI, GRP * D], F32)
                                  nc.tensor.transpose(
                                      ps2[:],
                                      obuf[:, gslot * GRP * NNEI + k * NNEI :
                                           gslot * GRP * NNEI + (k + 1) * NNEI],
                                      id12[:],
                                  )
                                  nc.vector.tensor_copy(
                                      wtile[:, (gslot * GRP + k) * D :
                                            (gslot * GRP + k + 1) * D],
                                      ps2[:, D * k : D * (k + 1)],
                                  )
                  if no_out or no_copy or no_mm:
                      continue
                  if b % OB == OB - 1:
                      if wide:
                          n_lo = (b - OB + 1) * NB
                          nn = OB * NB
                          getattr(nc, out_eng).dma_start(
                              out_d[:, n_lo : n_lo + nn, :],
                              wtile[:].rearrange("p (n d) -> p n d", n=nn),
                          )
                      else:
                          g0 = (b - OB + 1) * GPB
                          ng = OB * GPB
                          for k in range(GRP):
                              # diagonal block k of each group: SBUF partitions
                              # 3k..3k+3, free columns g*512 + 128k .. +128.
                              # dst out_d[k] is contiguous over g per (k,d) row.
                              src_k = obuf[D * k : D * (k + 1), :].rearrange(
                                  "p (g x) -> p g x", g=ng
                              )[:, :, k * NNEI : (k + 1) * NNEI]
                              getattr(nc, out_eng).dma_start(
                                  out_d[k, :, g0 : g0 + ng, :], src_k
                              )
    _split_excess_waits(nc)
    return nc


def make_shards(AA, h2, w):
    """Host-side data movement: shard + relayout inputs for the 8 cores."""
    AA4 = np.ascontiguousarray(AA, dtype=np.float32).reshape(NTOT, NNEI, NNEI, NH)
    h24 = np.ascontiguousarray(h2, dtype=np.float32).reshape(NTOT, NNEI, D)
    w = np.asarray(w, dtype=np.float32)

    # WS: [j, h] = w[h]  (partition replication only)
    ws = np.ascontiguousarray(np.broadcast_to(w[:, 0], (NNEI, NH)))

    in_maps = []
    for c in range(NCORES):
        aa_c = AA4[c * NPC : (c + 1) * NPC]             # [512, i, j, h]
        blk = aa_c.reshape(NBLK, NB, NNEI, NNEI, NH)    # [b, nb, i, j, h]
        aat = np.ascontiguousarray(blk.transpose(0, 3, 4, 1, 2))  # [b, j, h, nb, i]

        h2_c = h24[c * NPC : (c + 1) * NPC]             # [n, j, d]
        h1t = np.ascontiguousarray(h2_c.transpose(1, 0, 2))   # [j, n, d]

        in_maps.append(
            {
                "AAT": aat,
                "H1": h1t,
                "WS": ws,
                "ID12": np.eye(4 * D, dtype=np.float32),
                "tick": np.zeros((128, 8), np.float32),
            }
        )
    return in_maps


def assemble_output(results):
    """kd: [core][4, D, NGRP, NNEI]; wide: [core][NNEI, NPC, D] -> full"""
    outs = []
    for c in range(NCORES):
        o = results[c]["OUT"]
        if o.ndim == 3:                                  # wide: [128(i), n, 3]
            outs.append(o.transpose(1, 0, 2))            # [NPC, NNEI, D]
        else:                                            # kd: [4, 3, NGRP, 128]
            ngrp = o.shape[2]
            v = o.transpose(2, 0, 3, 1)                  # [NGRP, 4, 128, 3]
            outs.append(v.reshape(ngrp * 4, NNEI, D))    # [NPC, NNEI, D]
    full = np.concatenate(outs, axis=0)                  # [4096, 128, 3]
    return np.ascontiguousarray(full.reshape(NF, NLOC, NNEI, D))


_NC_CACHE = {}


def _get_nc():
    if "nc" not in _NC_CACHE:
        _NC_CACHE["nc"] = build_nc()
    return _NC_CACHE["nc"]


def kernel(AA, h2, w):
    from concourse.bass_utils import run_bass_kernel_spmd

    nc = _get_nc()
    in_maps = make_shards(AA, h2, w)
    res = run_bass_kernel_spmd(nc, in_maps, list(range(NCORES)))
    return assemble_output(res.results)


# ---------------------------------------------------------------------------
# Timing support (used by test.py, not by the grading path)
# ---------------------------------------------------------------------------

def make_runner(nc):
    """Compile `nc` into a reusable 8-core callable, mirroring
    bass2jax.run_bass_via_pjrt exactly (incl. output-buffer donation).
    Returns run(in_maps) -> (wall_seconds, results)."""
    import jax
    from jax.sharding import Mesh, PartitionSpec
    from jax.experimental.shard_map import shard_map
    from concourse import bass2jax
    from concourse.bass2jax import _bass_exec_p, partition_id_tensor

    bass2jax.install_neuronx_cc_hook()

    in_names, out_names, out_avals, zero_outs = [], [], [], []
    partition_name = nc.partition_id_tensor.name if nc.partition_id_tensor else None
    for alloc in nc.m.functions[0].allocations:
        if not isinstance(alloc, mybir.MemoryLocationSet):
            continue
        name = alloc.memorylocations[0].name
        if alloc.kind == "ExternalInput":
            if name != partition_name:
                in_names.append(name)
        elif alloc.kind == "ExternalOutput":
            out_names.append(name)
            shape = tuple(alloc.tensor_shape)
            dtype = mybir.dt.np(alloc.dtype)
            out_avals.append(jax.core.ShapedArray(shape, dtype))
            zero_outs.append(np.zeros(shape, dtype))
    n_params = len(in_names)
    all_in_names = tuple(in_names) + tuple(out_names) + \
        ((partition_name,) if partition_name else ())
    donate = tuple(range(n_params, n_params + len(out_names)))

    def _body(*args):
        operands = list(args)
        if partition_name is not None:
            operands.append(partition_id_tensor())
        outs = _bass_exec_p.bind(
            *operands,
            out_avals=tuple(out_avals),
            in_names=all_in_names,
            out_names=tuple(out_names),
            lowering_input_output_aliases=(),
            sim_require_finite=True,
            sim_require_nnan=True,
            nc=nc,
        )
        return tuple(outs)

    devices = jax.devices()[:NCORES]
    mesh = Mesh(np.asarray(devices), ("core",))
    in_specs = (PartitionSpec("core"),) * (n_params + len(out_names))
    out_specs = (PartitionSpec("core"),) * len(out_names)
    fn = jax.jit(
        shard_map(_body, mesh=mesh, in_specs=in_specs, out_specs=out_specs,
                  check_rep=False),
        donate_argnums=donate,
        keep_unused=True,
    )

    state = {}

    def run(in_maps, iters=1):
        """Returns (list_of_wall_seconds, results_of_last_iter).

        Big inputs are device-put once and cached; the donated zero output
        buffers are re-created per call.
        """
        import jax
        sharding = jax.sharding.NamedSharding(mesh, PartitionSpec("core"))
        key = id(in_maps)
        if state.get("key") != key:
            per_core = [[np.asarray(m[nm]) for nm in in_names] for m in in_maps]
            concat_in = [
                np.concatenate([per_core[c][i] for c in range(NCORES)], axis=0)
                for i in range(n_params)
            ]
            state["din"] = [jax.device_put(a, sharding) for a in concat_in]
            jax.block_until_ready(state["din"])
            state["key"] = key
        din = state["din"]

        def fresh_zeros():
            z = [np.zeros((NCORES * z0.shape[0], *z0.shape[1:]), z0.dtype)
                 for z0 in zero_outs]
            dz = [jax.device_put(a, sharding) for a in z]
            jax.block_until_ready(dz)
            return dz

        out = fn(*din, *fresh_zeros())
        jax.block_until_ready(out)  # warm-up
        walls = []
        for _ in range(iters):
            dz = fresh_zeros()
            t0 = time.perf_counter()
            out = fn(*din, *dz)
            jax.block_until_ready(out)
            walls.append(time.perf_counter() - t0)
        results = [
            {nm: np.asarray(out[i]).reshape(NCORES, *out_avals[i].shape)[c]
             for i, nm in enumerate(out_names)}
            for c in range(NCORES)
        ]
        return walls, results

    return run

